# revision 11
# baseline (speedup 1.0000x reference)
"""Trainium2 Bass kernel for fused multi-head attention + out-proj + LayerNorm.

Problem shapes (hardcoded):
  q, k, v: [B=4, N=2048, D=1024] fp32, W_out: [1024, 1024], g: [1024]
  NUM_HEADS=16, HEAD_DIM=64, LN_EPS=1e-5

Sharding over 8 NeuronCores: core c handles batch c//2, query rows
(c%2)*1024 ... +1024.  Each core sees all keys/values of its batch, so the
cores are fully independent (no collectives).

Per-core dataflow (fp16 matmul operands, fp32 PSUM accumulation):
  - DMA-cast q/k/v/W_out fp32->fp16 (SWDGE), xbar-DMA-transpose q/k/W tiles
    so the contraction dim sits on partitions.
  - Per head pair: S^T[m,n] = K^T Q (row-packed 2 heads), exp on ScalarE
    (softmax scale folded into the activation's scale, no max subtraction --
    logits are ~N(0,1) so exp is safe), row-sums via ones-matmul, P^T V
    col-packed into O^T[dv,n] layout, normalize by 1/rowsum.
  - Projection uses the O^T tiles directly as stationary operands; LayerNorm
    (bn_stats) + g on the [n, d_out] result; DMA out fp32.
"""

import sys

for _p in ("/opt/trn_rl_repo",):
    if _p not in sys.path:
        sys.path.insert(0, _p)

from contextlib import ExitStack

import numpy as np

import concourse.bass as bass
import concourse.bacc as bacc
import concourse.tile as tile
from concourse import mybir
from concourse.bass_utils import run_bass_kernel_spmd

F16 = mybir.dt.float16
F32 = mybir.dt.float32

B = 4
N = 2048
D = 1024
H = 16
HD = 64
LN_EPS = 1e-5
SCALE = HD ** -0.5

NQ = 1024          # query rows per core
M = 2048           # keys per core
MB = M // 128      # 16 m-blocks
NB = NQ // 128     # 8 n-blocks
HPAIRS = H // 2    # 8 head pairs
DB = D // 128      # 8 feature blocks

Exp = mybir.ActivationFunctionType.Exp
Sqrt = mybir.ActivationFunctionType.Sqrt
Alu = mybir.AluOpType


def _emit(tc: tile.TileContext, ctx: ExitStack, q, k, v, w, g, out):
    nc = tc.nc

    persist = ctx.enter_context(tc.tile_pool(name="persist", bufs=1))
    ldpool = ctx.enter_context(tc.tile_pool(name="ld", bufs=1))

    # ---- constants ----
    # [128, 64] of ones: rowsum matmuls use M=64 so the per-n rowsum is
    # replicated across all 64 output partitions of the head (the later
    # normalization then needs no cross-partition broadcast).
    ones16 = persist.tile([128, 64], F16, tag="ones")
    nc.vector.memset(ones16[:], 1.0)
    eps_t = persist.tile([128, 1], F32, tag="eps")
    nc.vector.memset(eps_t[:], LN_EPS)
    gbc = persist.tile([128, D], F32, tag="gbc")
    g_ap = g.ap()
    nc.gpsimd.dma_start(
        out=gbc[:], in_=bass.AP(tensor=g_ap.tensor, offset=0, ap=[[0, 128], [1, D]])
    )

    # ---- load + cast + transpose ----
    # Natural-layout fp16 copies (one big SWDGE cast DMA each), then xbar
    # transposes into [feature, row] layouts.
    # Transposed layouts as single 3D tensors: one xbar-transpose call per
    # 128-row slab block-transposes all 8 slices at once (out[:, j, :] =
    # T(in[:, 128j:128j+128])).
    QT = persist.tile([128, HPAIRS, NQ], F16, tag="qt")
    KT = persist.tile([128, HPAIRS, M], F16, tag="kt")
    WT = persist.tile([128, DB, D], F16, tag="wt")
    AT = persist.tile([128, DB, NQ], F16, tag="at")
    V16 = persist.tile([128, MB, D], F16, tag="v16")

    k16 = ldpool.tile([128, MB, D], F16, tag="k16")
    nc.gpsimd.dma_start(
        out=k16[:], in_=k.ap().rearrange("(mb p) d -> p mb d", p=128)
    )
    for mb in range(MB):
        nc.sync.dma_start(
            out=KT[:, :, mb * 128 : (mb + 1) * 128],
            in_=k16[:, mb, :],
            transpose=True,
        )

    q16 = ldpool.tile([128, NB, D], F16, tag="q16")
    nc.gpsimd.dma_start(
        out=q16[:], in_=q.ap().rearrange("(nb p) d -> p nb d", p=128)
    )
    for nb in range(NB):
        nc.scalar.dma_start(
            out=QT[:, :, nb * 128 : (nb + 1) * 128],
            in_=q16[:, nb, :],
            transpose=True,
        )

    nc.gpsimd.dma_start(
        out=V16[:], in_=v.ap().rearrange("(mb p) d -> p mb d", p=128)
    )

    w16 = ldpool.tile([128, DB, D], F16, tag="w16")
    nc.gpsimd.dma_start(
        out=w16[:], in_=w.ap().rearrange("(ob p) d -> p ob d", p=128)
    )
    for ob in range(DB):
        nc.scalar.dma_start(
            out=WT[:, :, ob * 128 : (ob + 1) * 128],
            in_=w16[:, ob, :],
            transpose=True,
        )

    # ---- attention ----
    with (
        tc.tile_pool(name="pst", bufs=2, space="PSUM") as pst,
        tc.tile_pool(name="pot", bufs=1, space="PSUM") as pot,
        tc.tile_pool(name="prs", bufs=1, space="PSUM") as prs,
        tc.tile_pool(name="ptp", bufs=4) as ptp,
        tc.tile_pool(name="misc", bufs=2) as misc,
    ):
        for hp in range(HPAIRS):
            h0, h1 = 2 * hp, 2 * hp + 1
            OT = pot.tile([128, NQ], F32, tag="ot")
            RS = prs.tile([128, NQ], F32, tag="rs")
            for mb in range(MB):
                kslice = slice(mb * 128, (mb + 1) * 128)
                ST0 = pst.tile([128, NQ], F32, tag="st")
                ST1 = pst.tile([128, NQ], F32, tag="st")
                for sti, po in ((ST0, 0), (ST1, 64)):
                    for c in (0, 512):
                        nc.tensor.matmul(
                            sti[:, c : c + 512],
                            lhsT=KT[po : po + 64, hp, kslice],
                            rhs=QT[po : po + 64, hp, c : c + 512],
                            start=True,
                            stop=True,
                            tile_position=(po, 0),
                        )
                PT0 = ptp.tile([128, NQ], F16, tag="pt")
                PT1 = ptp.tile([128, NQ], F16, tag="pt")
                nc.scalar.activation(out=PT0[:], in_=ST0[:], func=Exp, scale=SCALE)
                nc.scalar.activation(out=PT1[:], in_=ST1[:], func=Exp, scale=SCALE)
                st = mb == 0
                sp = mb == MB - 1
                for c in (0, 512):
                    nc.tensor.matmul(
                        RS[0:64, c : c + 512],
                        lhsT=ones16[:, 0:64],
                        rhs=PT0[:, c : c + 512],
                        start=st,
                        stop=sp,
                        tile_position=(0, 0),
                    )
                    nc.tensor.matmul(
                        RS[64:128, c : c + 512],
                        lhsT=ones16[:, 0:64],
                        rhs=PT1[:, c : c + 512],
                        start=st,
                        stop=sp,
                        tile_position=(0, 64),
                    )
                    nc.tensor.matmul(
                        OT[0:64, c : c + 512],
                        lhsT=V16[:, mb, h0 * HD : (h0 + 1) * HD],
                        rhs=PT0[:, c : c + 512],
                        start=st,
                        stop=sp,
                        tile_position=(0, 0),
                    )
                    nc.tensor.matmul(
                        OT[64:128, c : c + 512],
                        lhsT=V16[:, mb, h1 * HD : (h1 + 1) * HD],
                        rhs=PT1[:, c : c + 512],
                        start=st,
                        stop=sp,
                        tile_position=(0, 64),
                    )
            rbc = misc.tile([128, NQ], F32, tag="rbc")
            nc.vector.reciprocal(out=rbc[:], in_=RS[:])
            nc.vector.tensor_mul(AT[:, hp, :], OT[:], rbc[:])

    # ---- projection + layernorm ----
    sdim = nc.vector.BN_STATS_DIM
    adim = nc.vector.BN_AGGR_DIM
    with (
        tc.tile_pool(name="ppo", bufs=2, space="PSUM") as ppo,
        tc.tile_pool(name="lnp", bufs=3) as lnp,
    ):
        for nb in range(NB):
            nslice = slice(nb * 128, (nb + 1) * 128)
            PO = ppo.tile([128, D], F32, tag="po")
            for oc in (0, 512):
                for db in range(DB):
                    nc.tensor.matmul(
                        PO[:, oc : oc + 512],
                        lhsT=AT[:, db, nslice],
                        rhs=WT[:, db, oc : oc + 512],
                        start=db == 0,
                        stop=db == DB - 1,
                    )
            stats = lnp.tile([128, 2, sdim], F32, tag="stats")
            nc.vector.bn_stats(out=stats[:, 0, :], in_=PO[:, 0:512])
            nc.vector.bn_stats(out=stats[:, 1, :], in_=PO[:, 512:1024])
            mv = lnp.tile([128, adim], F32, tag="mv")
            nc.vector.bn_aggr(out=mv[:], in_=stats[:])
            rstd = lnp.tile([128, 1], F32, tag="rstd")
            nc.scalar.activation(
                out=rstd[:], in_=mv[:, 1:2], func=Sqrt, bias=eps_t[:], scale=1.0
            )
            nc.vector.reciprocal(out=rstd[:], in_=rstd[:])
            on = lnp.tile([128, D], F32, tag="on")
            nc.vector.tensor_scalar(
                out=on[:],
                in0=PO[:],
                scalar1=mv[:, 0:1],
                scalar2=rstd[:],
                op0=Alu.subtract,
                op1=Alu.mult,
            )
            on2 = lnp.tile([128, D], F32, tag="on2")
            nc.vector.tensor_mul(on2[:], on[:], gbc[:])
            nc.sync.dma_start(out=out[nslice, :], in_=on2[:])


def _build_nc(repeats=1):
    nc = bacc.Bacc("TRN2", target_bir_lowering=False, debug=False)
    q = nc.dram_tensor("q", [NQ, D], F32, kind="ExternalInput")
    k = nc.dram_tensor("k", [M, D], F32, kind="ExternalInput")
    v = nc.dram_tensor("v", [M, D], F32, kind="ExternalInput")
    w = nc.dram_tensor("W_out", [D, D], F32, kind="ExternalInput")
    g = nc.dram_tensor("g", [D], F32, kind="ExternalInput")
    out = nc.dram_tensor("out", [NQ, D], F32, kind="ExternalOutput")
    with ExitStack() as ctx:
        tc = ctx.enter_context(tile.TileContext(nc))
        for _ in range(repeats):
            with ExitStack() as rep_ctx:
                _emit(tc, rep_ctx, q, k, v, w, g, out.ap())
    nc.finalize()
    return nc


_NC_CACHE = None


def _get_nc():
    global _NC_CACHE
    if _NC_CACHE is None:
        _NC_CACHE = _build_nc()
    return _NC_CACHE


def _make_in_maps(q, k, v, W_out, g):
    in_maps = []
    for c in range(8):
        b, half = divmod(c, 2)
        rows = slice(half * NQ, (half + 1) * NQ)
        in_maps.append(
            {
                "q": np.ascontiguousarray(q[b, rows, :]),
                "k": np.ascontiguousarray(k[b]),
                "v": np.ascontiguousarray(v[b]),
                "W_out": np.ascontiguousarray(W_out),
                "g": np.ascontiguousarray(g),
            }
        )
    return in_maps


def run_sharded(q, k, v, W_out, g, **kwargs):
    """Run the SPMD kernel; returns (full_output, BassKernelResults)."""
    nc = _get_nc()
    in_maps = _make_in_maps(q, k, v, W_out, g)
    res = run_bass_kernel_spmd(nc, in_maps, core_ids=list(range(8)), **kwargs)
    full = np.empty((B, N, D), dtype=np.float32)
    for c in range(8):
        b, half = divmod(c, 2)
        full[b, half * NQ : (half + 1) * NQ, :] = res.results[c]["out"]
    return full, res


def kernel(q, k, v, W_out, g):
    q = np.asarray(q, dtype=np.float32)
    k = np.asarray(k, dtype=np.float32)
    v = np.asarray(v, dtype=np.float32)
    W_out = np.asarray(W_out, dtype=np.float32)
    g = np.asarray(g, dtype=np.float32)
    full, _ = run_sharded(q, k, v, W_out, g)
    return full
